# revision 9
# baseline (speedup 1.0000x reference)
"""Self-contained Trainium2 Bass kernel for a 2-layer GAT (GATConv) over a
random graph, distributed over 8 NeuronCores.

Strategy (graph/data parallel, dst-ownership):
  - Nodes are partitioned N/8 per core. Each core owns the edges whose
    destination is local (plus local self-loops), so the segment softmax and
    the scatter-add aggregation stay device-local.
  - Each core redundantly computes the projected feature table
    xse = x @ [W1 | asrc_eff | adst_eff]  for ALL nodes (cheap on the PE,
    avoids an all-gather of the wide layer-1 features), writes it to HBM and
    uses the SWDGE dma_gather to fetch per-edge source rows.
  - Per-edge attention weights w = exp(leaky_relu(asrc[src] + adst[dst]))
    (the max-subtraction of the reference softmax is skipped - it is
    mathematically a no-op and the scores are small enough for fp32 exp).
  - Weighted features (and the softmax denominators, fused as extra columns)
    are accumulated with dma_scatter_add into a per-core local table.
  - Between the layers only the narrow hse = [h@W2 | asrc2 | adst2] table is
    exchanged with an AllGather.

dma_gather indices are int16, so source gathers are split in two windows
(node < 32768 / node >= 32768). All per-core edge lists are padded to common
sizes with dummy edges (src 0 / dst -> trash row) so one SPMD program fits
every core.
"""

import os
import sys

for _p in ("/opt/trn_rl_repo", "/root/.axon_site/_ro/trn_rl_repo"):
    if os.path.isdir(_p) and _p not in sys.path:
        sys.path.insert(0, _p)

import numpy as np

# ---------------------------------------------------------------------------
# Problem constants (hardcoded per the task contract). The small-scale sim
# test monkeypatches these module globals, so read them at call time only.
# ---------------------------------------------------------------------------
N = 50000
IN_CH = 128
HID = 32
HEADS = 8
H1C = HEADS * HID  # 256
OUT_CH = 64
NEG = 0.2
NCORES = 8
SPLIT = 32768   # int16 gather index limit
CHUNK = 1024    # edges per gather/scatter op (dma_gather crashes at >=1536 idxs)
XSE_W = 320     # xse row width (256 feat + 8 asrc + 8 adst + pad) = 1280B
HSE_W = 128     # hse row width (64 hs + asrc2 + adst2 + pad) = 512B

P = 128

_PROG_CACHE = {}
TRACE = False
TRACE_KW = {}
LAST_RESULTS = None


def _cfg():
    nloc = N // NCORES
    nrows = ((nloc + 1 + P - 1) // P) * P  # accum/score table rows
    return dict(n=N, nloc=nloc, trash=nloc, nrows=nrows, split=SPLIT,
                chunk=CHUNK, ncores=NCORES)


def _wrap_idx(arr):
    """[L] int array -> [128, L//16] int16 SWDGE wrapped layout.

    unwrapped[s*16 + p] = wrapped[p, s]; replicated across the 8 Q7 cores
    (partition groups of 16)."""
    L = arr.shape[0]
    assert L % 16 == 0
    w = np.ascontiguousarray(arr.reshape(L // 16, 16).T).astype(np.int16)
    return np.tile(w, (8, 1))


def _build_program(cfg, ne_lo, ne_hi):
    import concourse.bacc as bacc
    import concourse.tile as tile
    from concourse import mybir
    from concourse.masks import make_identity
    from contextlib import ExitStack

    f32 = mybir.dt.float32
    i16 = mybir.dt.int16
    AF = mybir.ActivationFunctionType

    n = cfg["n"]
    nloc = cfg["nloc"]
    nrows = cfg["nrows"]
    split = cfg["split"]
    chunk = cfg["chunk"]
    ncores = cfg["ncores"]

    ne = ne_lo + ne_hi
    nt = (n + P - 1) // P          # projection tiles
    nt_full = n // P               # full projection tiles
    tail_rows = n - nt_full * P
    ntl = (nloc + P - 1) // P      # local node tiles
    n_lo_rows = min(n, split)
    assert n > split
    ngrp = (nt + 15) // 16         # phase-A scatter groups (2048 nodes each)
    nnode_pad = ngrp * 2048

    nc = bacc.Bacc("TRN2", target_bir_lowering=False, debug=False,
                   num_devices=ncores)

    # ---------------- I/O ----------------
    x_in = nc.dram_tensor("x", [n, IN_CH], f32, kind="ExternalInput")
    w1_in = nc.dram_tensor("W1", [IN_CH, H1C], f32, kind="ExternalInput")
    asrc1_in = nc.dram_tensor("att_src1", [HEADS, HID], f32, kind="ExternalInput")
    adst1_in = nc.dram_tensor("att_dst1", [HEADS, HID], f32, kind="ExternalInput")
    bias1_in = nc.dram_tensor("bias1", [H1C], f32, kind="ExternalInput")
    w2_in = nc.dram_tensor("W2", [H1C, OUT_CH], f32, kind="ExternalInput")
    asrc2_in = nc.dram_tensor("att_src2", [1, OUT_CH], f32, kind="ExternalInput")
    adst2_in = nc.dram_tensor("att_dst2", [1, OUT_CH], f32, kind="ExternalInput")
    bias2_in = nc.dram_tensor("bias2", [OUT_CH], f32, kind="ExternalInput")
    srclo_in = nc.dram_tensor("src_lo", [P, ne_lo // 16], i16, kind="ExternalInput")
    srchi_in = nc.dram_tensor("src_hi", [P, ne_hi // 16], i16, kind="ExternalInput")
    dsti_in = nc.dram_tensor("dst_idx", [P, ne // 16], i16, kind="ExternalInput")
    locm_in = nc.dram_tensor("locmap", [P, nnode_pad // 16], i16,
                             kind="ExternalInput")
    out_ext = nc.dram_tensor("out", [nloc, OUT_CH], f32, kind="ExternalOutput")

    # ---------------- internal DRAM ----------------
    xse_d = nc.dram_tensor("xse", [n, XSE_W], f32)
    adst1_d = nc.dram_tensor("adst1t", [nrows, 64], f32)
    accum1_d = nc.dram_tensor("accum1", [nrows, XSE_W], f32)
    hseloc_d = nc.dram_tensor("hseloc", [nloc, HSE_W], f32)
    hsefull_d = nc.dram_tensor("hsefull", [ncores * nloc, HSE_W], f32,
                               addr_space="Shared")
    adst2_d = nc.dram_tensor("adst2t", [nrows, 64], f32)
    accum2_d = nc.dram_tensor("accum2", [nrows, HSE_W], f32)

    with tile.TileContext(nc) as tc, ExitStack() as ctx:
        const_p = ctx.enter_context(tc.tile_pool(name="const", bufs=1))
        idx_p = ctx.enter_context(tc.tile_pool(name="idx", bufs=1))
        projx_p = ctx.enter_context(tc.tile_pool(name="projx", bufs=3))
        projsb_p = ctx.enter_context(tc.tile_pool(name="projsb", bufs=3))
        stage_p = ctx.enter_context(tc.tile_pool(name="stage", bufs=3))
        adstg_p = ctx.enter_context(tc.tile_pool(name="adstg", bufs=2))
        gath_p = ctx.enter_context(tc.tile_pool(name="gath", bufs=3))
        adt_p = ctx.enter_context(tc.tile_pool(name="adt", bufs=3))
        sc_p = ctx.enter_context(tc.tile_pool(name="sc", bufs=4))
        fin_p = ctx.enter_context(tc.tile_pool(name="fin", bufs=3))
        ps_a = ctx.enter_context(tc.tile_pool(name="ps_a", bufs=2, space="PSUM"))
        ps_b = ctx.enter_context(tc.tile_pool(name="ps_b", bufs=2, space="PSUM"))

        # ================= setup =================
        ident = const_p.tile([P, P], f32)
        make_identity(nc, ident[:])

        # idx arrays to SBUF
        srclo_sb = idx_p.tile([P, ne_lo // 16], i16)
        nc.sync.dma_start(srclo_sb[:], srclo_in[:])
        srchi_sb = idx_p.tile([P, ne_hi // 16], i16)
        nc.sync.dma_start(srchi_sb[:], srchi_in[:])
        dsti_sb = idx_p.tile([P, ne // 16], i16)
        nc.sync.dma_start(dsti_sb[:], dsti_in[:])
        locm_sb = idx_p.tile([P, nnode_pad // 16], i16)
        nc.sync.dma_start(locm_sb[:], locm_in[:])

        # W1ext = [W1 | asrc_eff | adst_eff]  [128, 272]
        w1ext = const_p.tile([P, H1C + 16], f32)
        nc.sync.dma_start(w1ext[:, 0:H1C], w1_in[:])

        # att (src|dst) stacked [16, 32] -> attT [32, 16]
        attcat = const_p.tile([16, HID], f32)
        nc.sync.dma_start(attcat[0:HEADS, :], asrc1_in[:])
        nc.sync.dma_start(attcat[HEADS:16, :], adst1_in[:])
        attT_ps = ps_a.tile([HID, 16], f32, tag="pst")
        nc.tensor.transpose(attT_ps[:], attcat[:], ident[0:16, 0:16])
        attT = const_p.tile([HID, 16], f32)
        nc.vector.tensor_copy(attT[:], attT_ps[:])

        # block-diagonal B [256, 16] as two tiles [128, 16]
        b0 = const_p.tile([P, 16], f32)
        b1 = const_p.tile([P, 16], f32)
        nc.vector.memset(b0[:], 0.0)
        nc.vector.memset(b1[:], 0.0)
        for h in range(HEADS):
            bt = b0 if h < 4 else b1
            r0 = (h % 4) * HID
            nc.sync.dma_start(bt[r0:r0 + HID, h:h + 1], attT[0:HID, h:h + 1])
            nc.sync.dma_start(bt[r0:r0 + HID, 8 + h:9 + h],
                              attT[0:HID, 8 + h:9 + h])

        # eff = W1 @ B via W1T tiles
        eff_ps = ps_b.tile([P, 16], f32, tag="psm")
        for j in range(2):
            w1t_ps = ps_a.tile([P, P], f32, tag="pst")
            nc.tensor.transpose(w1t_ps[:], w1ext[:, j * P:(j + 1) * P], ident[:])
            w1t_sb = projsb_p.tile([P, P], f32, tag="tsb")
            nc.vector.tensor_copy(w1t_sb[:], w1t_ps[:])
            nc.tensor.matmul(eff_ps[:], lhsT=w1t_sb[:],
                             rhs=(b0 if j == 0 else b1)[:],
                             start=(j == 0), stop=(j == 1))
        nc.vector.tensor_copy(w1ext[:, H1C:H1C + 16], eff_ps[:])

        # W2 tiles [128, 64] x2
        w2a = const_p.tile([P, OUT_CH], f32)
        nc.sync.dma_start(w2a[:], w2_in[0:P, :])
        w2b = const_p.tile([P, OUT_CH], f32)
        nc.sync.dma_start(w2b[:], w2_in[P:H1C, :])

        # replicated row vectors
        def rep_row(src_ap, width, name):
            row = const_p.tile([1, width], f32, tag=f"row_{name}")
            nc.sync.dma_start(row[:], src_ap)
            rep = const_p.tile([P, width], f32, tag=f"rep_{name}")
            nc.gpsimd.partition_broadcast(rep[:], row[:])
            return rep

        bias1_rep = rep_row(bias1_in[None, :], H1C, "b1")
        bias2_rep = rep_row(bias2_in[None, :], OUT_CH, "b2")
        att2s_rep = rep_row(asrc2_in[:], OUT_CH, "a2s")
        att2d_rep = rep_row(adst2_in[:], OUT_CH, "a2d")

        # zero the DRAM accumulator / score tables
        zero_sb = const_p.tile([P, 2048], f32)
        nc.vector.memset(zero_sb[:], 0.0)

        def zero_dram(t):
            v = t[:].rearrange("(p r) c -> p (r c)", p=P)
            nco = v.shape[1]
            o = 0
            while o < nco:
                w = min(2048, nco - o)
                nc.sync.dma_start(v[:, o:o + w], zero_sb[:, 0:w])
                o += w

        zero_dram(accum1_d)
        zero_dram(accum2_d)
        zero_dram(adst1_d)
        zero_dram(adst2_d)

        # ================= phase A: projection =================
        adstg = None
        x_bulk = (x_in[0:nt_full * P, :]
                  .rearrange("(a p) c -> p a c", p=P)) if nt_full else None
        xse_bulk = (xse_d[0:nt_full * P, :]
                    .rearrange("(a p) c -> p a c", p=P)) if nt_full else None

        def proj_tile(t, xt_ap, xse4, slot):
            """xt_ap: [128, IN_CH] SBUF AP holding x rows of tile t (junk rows
            beyond the valid count are fine); writes xse4[:, slot, :]."""
            nonlocal adstg
            xT_ps = ps_a.tile([P, P], f32, tag="pst")
            nc.tensor.transpose(xT_ps[:], xt_ap, ident[:])
            xT_sb = projsb_p.tile([P, P], f32, tag="tsb")
            nc.any.tensor_copy(xT_sb[:], xT_ps[:])
            xse_ps = ps_b.tile([P, H1C + 16], f32, tag="psm")
            nc.tensor.matmul(xse_ps[:], lhsT=xT_sb[:], rhs=w1ext[:],
                             start=True, stop=True)
            nc.any.tensor_copy(xse4[:, slot, 0:H1C + 16], xse_ps[:])
            gg, jj = t // 16, t % 16
            if jj == 0:
                adstg = adstg_p.tile([P, 16, 64], f32, tag="adstg")
                nc.vector.memset(adstg[:], 0.0)
            nc.any.tensor_copy(adstg[:, jj, 0:HEADS],
                               xse_ps[:, H1C + 8:H1C + 16])
            if jj == 15 or t == nt - 1:
                nc.gpsimd.dma_scatter_add(
                    adst1_d[:], adstg[:], locm_sb[:, gg * 128:(gg + 1) * 128],
                    2048, 2048, 64)

        nt4 = (nt_full + 3) // 4
        for g4 in range(nt4):
            th = min(4, nt_full - g4 * 4)
            xt4 = projx_p.tile([P, 4, IN_CH], f32, tag="xt4")
            nc.sync.dma_start(xt4[:, 0:th, :], x_bulk[:, g4 * 4:g4 * 4 + th, :])
            xse4 = stage_p.tile([P, 4, XSE_W], f32, tag="xse4")
            nc.vector.memset(xse4[:, :, H1C + 16:XSE_W], 0.0)
            for a in range(th):
                proj_tile(g4 * 4 + a, xt4[:, a, :], xse4, a)
            nc.sync.dma_start(xse_bulk[:, g4 * 4:g4 * 4 + th, :],
                              xse4[:, 0:th, :])
        if tail_rows:
            xt1 = projx_p.tile([P, IN_CH], f32, tag="xt1")
            nc.vector.memset(xt1[:], 0.0)
            nc.sync.dma_start(xt1[0:tail_rows, :], x_in[nt_full * P:n, :])
            xse4 = stage_p.tile([P, 4, XSE_W], f32, tag="xse4")
            nc.vector.memset(xse4[:, :, H1C + 16:XSE_W], 0.0)
            proj_tile(nt_full, xt1[:], xse4, 0)
            nc.sync.dma_start(xse_d[nt_full * P:n, :],
                              xse4[0:tail_rows, 0, :])

        # ================= edge phases =================
        nch_lo = ne_lo // chunk
        nch_hi = ne_hi // chunk
        cpc = chunk // 128  # free rows per chunk tile
        ccol = chunk // 16  # idx cols per chunk

        def edge_phase(tab_lo, tab_hi, row_w, nfeat, wcol, accum_d,
                       adst_table, sc_w):
            """nfeat feature cols; w (and gathered asrc) lives at col wcol;
            sc_w score cols (heads)."""
            for c in range(nch_lo + nch_hi):
                if c < nch_lo:
                    idx_ap = srclo_sb[:, c * ccol:(c + 1) * ccol]
                    tab = tab_lo
                else:
                    ch = c - nch_lo
                    idx_ap = srchi_sb[:, ch * ccol:(ch + 1) * ccol]
                    tab = tab_hi
                gt = gath_p.tile([P, cpc, row_w], f32, tag="gath")
                nc.gpsimd.dma_gather(gt[:], tab, idx_ap, chunk, chunk, row_w)
                at = adt_p.tile([P, cpc, 64], f32, tag="adt")
                nc.gpsimd.dma_gather(at[:], adst_table,
                                     dsti_sb[:, c * ccol:(c + 1) * ccol],
                                     chunk, chunk, 64)
                s = sc_p.tile([P, cpc, 8], f32, tag="sc1")
                s2 = sc_p.tile([P, cpc, 8], f32, tag="sc2")
                nc.vector.tensor_add(s[:, :, 0:sc_w],
                                     gt[:, :, wcol:wcol + sc_w],
                                     at[:, :, 0:sc_w])
                nc.vector.tensor_scalar_mul(s2[:, :, 0:sc_w],
                                            s[:, :, 0:sc_w], NEG)
                nc.vector.tensor_max(s[:, :, 0:sc_w], s[:, :, 0:sc_w],
                                     s2[:, :, 0:sc_w])
                nc.scalar.activation(gt[:, :, wcol:wcol + sc_w],
                                     s[:, :, 0:sc_w], AF.Exp)
                cph = nfeat // sc_w  # channels per head
                g4d = gt[:, :, 0:nfeat].rearrange("p a (h c) -> p a h c",
                                                  c=cph)
                wb = gt[:, :, wcol:wcol + sc_w].to_broadcast(
                    [P, cpc, sc_w, cph])
                nc.vector.tensor_mul(g4d, g4d, wb)
                nc.gpsimd.dma_scatter_add(
                    accum_d[:], gt[:], dsti_sb[:, c * ccol:(c + 1) * ccol],
                    chunk, chunk, row_w)

        # ---- layer 1 ----
        edge_phase(xse_d[0:n_lo_rows, :], xse_d[split:n, :], XSE_W, H1C, H1C,
                   accum1_d, adst1_d[:], HEADS)

        # ================= phase F1: finalize layer 1 =================
        for t in range(ntl):
            rows = min(P, nloc - t * P)
            acc = fin_p.tile([P, XSE_W], f32, tag="acc1")
            nc.vector.memset(acc[:], 1.0)
            nc.sync.dma_start(acc[0:rows, :], accum1_d[t * P:t * P + rows, :])
            recip = sc_p.tile([P, 8], f32, tag="recip")
            nc.vector.reciprocal(recip[:], acc[:, H1C:H1C + 8])
            h = fin_p.tile([P, H1C], f32, tag="h")
            a3 = acc[:, 0:H1C].rearrange("p (h c) -> p h c", c=HID)
            h3 = h[:].rearrange("p (h c) -> p h c", c=HID)
            nc.vector.tensor_mul(h3, a3, recip[:].to_broadcast([P, HEADS, HID]))
            nc.vector.tensor_add(h[:], h[:], bias1_rep[:])
            # ELU = relu(h) + exp(min(h,0)) - 1
            tneg = fin_p.tile([P, H1C], f32, tag="tneg")
            nc.vector.tensor_scalar_min(tneg[:], h[:], 0.0)
            nc.scalar.activation(tneg[:], tneg[:], AF.Exp)
            nc.vector.tensor_scalar_max(h[:], h[:], 0.0)
            nc.vector.tensor_add(h[:], h[:], tneg[:])
            nc.vector.tensor_scalar_add(h[:], h[:], -1.0)
            # hs = h @ W2
            hs_ps = ps_b.tile([P, OUT_CH], f32, tag="psm")
            for j in range(2):
                hT_ps = ps_a.tile([P, P], f32, tag="pst")
                nc.tensor.transpose(hT_ps[:], h[:, j * P:(j + 1) * P], ident[:])
                hT_sb = projsb_p.tile([P, P], f32, tag="tsb")
                nc.any.tensor_copy(hT_sb[:], hT_ps[:])
                nc.tensor.matmul(hs_ps[:], lhsT=hT_sb[:],
                                 rhs=(w2a if j == 0 else w2b)[:],
                                 start=(j == 0), stop=(j == 1))
            hse = fin_p.tile([P, HSE_W], f32, tag="hse")
            nc.vector.memset(hse[:, OUT_CH + 2:HSE_W], 0.0)
            nc.any.tensor_copy(hse[:, 0:OUT_CH], hs_ps[:])
            tmp = fin_p.tile([P, OUT_CH], f32, tag="tmp64")
            nc.vector.tensor_mul(tmp[:], hse[:, 0:OUT_CH], att2s_rep[:])
            nc.vector.reduce_sum(hse[:, OUT_CH:OUT_CH + 1], tmp[:],
                                 axis=mybir.AxisListType.X)
            nc.vector.tensor_mul(tmp[:], hse[:, 0:OUT_CH], att2d_rep[:])
            nc.vector.reduce_sum(hse[:, OUT_CH + 1:OUT_CH + 2], tmp[:],
                                 axis=mybir.AxisListType.X)
            nc.sync.dma_start(hseloc_d[t * P:t * P + rows, :], hse[0:rows, :])
            nc.sync.dma_start(adst2_d[t * P:t * P + rows, 0:1],
                              hse[0:rows, OUT_CH + 1:OUT_CH + 2])

        # ================= AllGather =================
        nc.gpsimd.collective_compute(
            "AllGather", mybir.AluOpType.bypass,
            replica_groups=[list(range(ncores))],
            ins=[hseloc_d[:]], outs=[hsefull_d[:]])

        # ---- layer 2 ----
        edge_phase(hsefull_d[0:n_lo_rows, :], hsefull_d[split:n, :], HSE_W,
                   OUT_CH, OUT_CH, accum2_d, adst2_d[:], 1)

        # ================= phase F2 =================
        for t in range(ntl):
            rows = min(P, nloc - t * P)
            acc2 = fin_p.tile([P, HSE_W], f32, tag="acc2")
            nc.vector.memset(acc2[:], 1.0)
            nc.sync.dma_start(acc2[0:rows, :], accum2_d[t * P:t * P + rows, :])
            recip2 = sc_p.tile([P, 1], f32, tag="recip2")
            nc.vector.reciprocal(recip2[:], acc2[:, OUT_CH:OUT_CH + 1])
            o = fin_p.tile([P, OUT_CH], f32, tag="o")
            nc.vector.tensor_mul(o[:], acc2[:, 0:OUT_CH],
                                 recip2[:].to_broadcast([P, OUT_CH]))
            nc.vector.tensor_add(o[:], o[:], bias2_rep[:])
            nc.sync.dma_start(out_ext[t * P:t * P + rows, :], o[0:rows, :])

    nc.compile()
    return nc


def _dst_ranks(d_arr):
    """rank of each edge within its dst group (requires any order; computed
    via stable sort)."""
    order = np.argsort(d_arr, kind="stable")
    ds = d_arr[order]
    n = len(ds)
    if n == 0:
        return order, ds, np.zeros(0, np.int64)
    change = np.r_[True, ds[1:] != ds[:-1]]
    seg_start = np.maximum.accumulate(np.where(change, np.arange(n), 0))
    rank = np.arange(n) - seg_start
    return order, ds, rank


def _sched_group(s_arr, d_arr, nch, chunk, trash):
    """Assign edges to chunks so that no chunk repeats a (non-trash) dst.

    dma_scatter_add races on duplicate indices within one op, so each dst's
    edges are round-robined across chunks. Returns (src, dst) padded arrays of
    length nch*chunk, or None if nch is too small."""
    order, ds, rank = _dst_ranks(d_arr)
    ss = s_arr[order]
    if len(ds) and rank.max() >= nch:
        return None
    cid = (ds + rank) % nch if len(ds) else ds
    fill = np.bincount(cid, minlength=nch) if len(ds) else np.zeros(nch, int)
    if len(ds) and fill.max() > chunk:
        return None
    out_s = np.zeros(nch * chunk, np.int64)
    out_d = np.full(nch * chunk, trash, np.int64)
    if len(ds):
        ord2 = np.argsort(cid, kind="stable")
        cc = cid[ord2]
        within = np.arange(len(cc)) - np.searchsorted(cc, cc, side="left")
        slots = cc * chunk + within
        out_s[slots] = ss[ord2]
        out_d[slots] = ds[ord2]
    return out_s, out_d


def _prep_inputs(cfg, x, edge_index, W1, att_src1, att_dst1, bias1, W2,
                 att_src2, att_dst2, bias2):
    n = cfg["n"]
    nloc = cfg["nloc"]
    trash = cfg["trash"]
    split = cfg["split"]
    chunk = cfg["chunk"]
    ncores = cfg["ncores"]

    src = np.asarray(edge_index[0], dtype=np.int64)
    dst = np.asarray(edge_index[1], dtype=np.int64)
    loop = np.arange(n, dtype=np.int64)
    src = np.concatenate([src, loop])
    dst = np.concatenate([dst, loop])
    own = dst // nloc

    per_core = []
    max_lo = max_hi = 0
    for k in range(ncores):
        sel = own == k
        s_k = src[sel]
        d_k = dst[sel] - k * nloc
        lo = s_k < split
        slo, dlo = s_k[lo], d_k[lo]
        shi, dhi = s_k[~lo] - split, d_k[~lo]
        per_core.append((slo, dlo, shi, dhi))
        max_lo = max(max_lo, len(slo))
        max_hi = max(max_hi, len(shi))

    ne_lo = ((max_lo + chunk - 1) // chunk) * chunk
    ne_hi = ((max_hi + chunk - 1) // chunk) * chunk

    # raise the chunk counts until the collision-free schedule fits every core
    def solve(group_idx, ne):
        nch = ne // chunk
        while True:
            scheds = []
            ok = True
            for k in range(ncores):
                slo, dlo, shi, dhi = per_core[k]
                s_a, d_a = (slo, dlo) if group_idx == 0 else (shi, dhi)
                r = _sched_group(s_a, d_a, nch, chunk, trash)
                if r is None:
                    ok = False
                    break
                scheds.append(r)
            if ok:
                return nch * chunk, scheds
            nch += 1

    ne_lo, scheds_lo = solve(0, ne_lo)
    ne_hi, scheds_hi = solve(1, ne_hi)

    nt = (n + P - 1) // P
    nnode_pad = ((nt + 15) // 16) * 2048

    common = {
        "x": np.ascontiguousarray(np.asarray(x, dtype=np.float32)),
        "W1": np.ascontiguousarray(np.asarray(W1, dtype=np.float32)),
        "att_src1": np.ascontiguousarray(np.asarray(att_src1, np.float32)),
        "att_dst1": np.ascontiguousarray(np.asarray(att_dst1, np.float32)),
        "bias1": np.ascontiguousarray(np.asarray(bias1, np.float32)),
        "W2": np.ascontiguousarray(np.asarray(W2, np.float32)),
        "att_src2": np.ascontiguousarray(np.asarray(att_src2, np.float32)),
        "att_dst2": np.ascontiguousarray(np.asarray(att_dst2, np.float32)),
        "bias2": np.ascontiguousarray(np.asarray(bias2, np.float32)),
    }
    nodes = np.arange(nnode_pad, dtype=np.int64)
    in_maps = []
    for k in range(ncores):
        slo_p, dlo_p = scheds_lo[k]
        shi_p, dhi_p = scheds_hi[k]
        locmap = np.where((nodes >= k * nloc) & (nodes < (k + 1) * nloc),
                          nodes - k * nloc, trash)
        m = dict(common)
        m["src_lo"] = _wrap_idx(slo_p)
        m["src_hi"] = _wrap_idx(shi_p)
        m["dst_idx"] = _wrap_idx(np.concatenate([dlo_p, dhi_p]))
        m["locmap"] = _wrap_idx(locmap)
        in_maps.append(m)
    return ne_lo, ne_hi, in_maps


def kernel(x, edge_index, W1, att_src1, att_dst1, bias1, W2, att_src2,
           att_dst2, bias2):
    global LAST_RESULTS
    from concourse.bass_utils import run_bass_kernel_spmd

    cfg = _cfg()
    ne_lo, ne_hi, in_maps = _prep_inputs(
        cfg, x, edge_index, W1, att_src1, att_dst1, bias1, W2, att_src2,
        att_dst2, bias2)
    key = (cfg["n"], ne_lo, ne_hi)
    if key not in _PROG_CACHE:
        _PROG_CACHE[key] = _build_program(cfg, ne_lo, ne_hi)
    nc = _PROG_CACHE[key]
    res = run_bass_kernel_spmd(nc, in_maps, list(range(cfg["ncores"])),
                               trace=TRACE, **TRACE_KW)
    LAST_RESULTS = res
    out = np.concatenate([res.results[k]["out"] for k in range(cfg["ncores"])],
                         axis=0)
    return out.astype(np.float32)
